# revision 17
# baseline (speedup 1.0000x reference)
"""GCN (4-layer, BA-Shapes-style) Trainium2 kernel, 8 NeuronCores.

Strategy (graph/data parallel, per sharding hint):
  - Nodes are sharded into 8 contiguous ranges (6250 nodes/core). Each core
    owns the aggregation for its node range.
  - Per layer: each core computes hs = dinv * (x @ W) for its own rows,
    AllGathers hs into a replicated DRAM table, then PULL-aggregates: for
    every owned destination node, dma_gather fetches the hs rows of its
    in-neighbors (+ self loop) and a Vector-engine segmented reduce sums
    them.  out = leaky_relu(dinv * agg + b).
  - GCN normalization deg^-1/2 factorizes to a row scale before the gather
    and a row scale after the reduce, so no per-edge weights are needed.
  - Head: concepts = softmax(h)/max(softmax(h)) == exp(h - rowmax(h));
    logits = concepts @ Wl + bl.

Host-side work is restricted to integer index preprocessing (edge lists,
degree counts, permutations, padding plans) and input/output layout
(transpose / permutation of rows); all floating-point math runs on device.

Index-layout details:
  - dma_gather indices are int16, so rows are fetched as aligned PAIRS
    (elem covers nodes 2j and 2j+1; pair id <= 25064 fits int16).  A
    per-slot parity mask selects the wanted half via copy_predicated.
  - Destinations are bucketed 128-at-a-time by degree rank; every bucket is
    padded to a cross-core-uniform slot count D[b] so all 8 cores share one
    NEFF.  Pad slots gather a zeroed row (contributing 0 to the sum).
"""

import numpy as np
import ml_dtypes

# ----------------------------------------------------------------- config

N_CORES = 8
N, E = 50000, 800000
F_IN, HID, CDIM, KDIM = 128, 64, 32, 4

SCAP = 64          # max slot-columns per dma_gather call (num_idxs = 128*SCAP)
GATHER_BUFS = 3
USE_BF16 = True    # table/gather/select dtype (reduce always accumulates fp32)


def _cfg(n, e, n_cores, use_bf16=USE_BF16, scap=SCAP):
    n_loc = n // n_cores
    nb = (n_loc + 127) // 128
    return dict(
        N=n, E=e, NC=n_cores, n_loc=n_loc, NB=nb, n_pad=nb * 128,
        TROWS=n + 128,            # extra zeroed rows for pad slots
        use_bf16=use_bf16, scap=scap,
    )


# ----------------------------------------------------- host preprocessing

def _preprocess(cfg, src, dst):
    """Integer-only index preprocessing.

    Returns (cfg additions, per-core arrays):
      perm_old[new] : global node permutation (core-range, degree-sorted)
      D[b]          : uniform bucket slot counts
      chunks        : [(b0, b1, off0, S)] gather-call partition
      per core: idx16 [128, 8*SD] int16, maskL [128, SD], deg [128, NB] f32
    """
    n, nc_, n_loc, nb = cfg["N"], cfg["NC"], cfg["n_loc"], cfg["NB"]
    src = np.asarray(src, dtype=np.int64)
    dst = np.asarray(dst, dtype=np.int64)

    indeg = np.bincount(dst, minlength=n)
    deg = indeg + 1  # self loop

    # permutation: within each core's contiguous range, sort nodes by degree
    perm_old = np.empty(n, dtype=np.int64)   # new -> old
    for c in range(nc_):
        lo = c * n_loc
        rng = np.arange(lo, lo + n_loc)
        order = np.argsort(deg[rng], kind="stable")
        perm_old[lo:lo + n_loc] = rng[order]
    new_of_old = np.empty(n, dtype=np.int64)
    new_of_old[perm_old] = np.arange(n)

    src_n = new_of_old[src]
    dst_n = new_of_old[dst]
    deg_new = deg[perm_old]                  # degree in new id order

    # bucket profile: D[b] = max over cores of bucket max degree
    D = np.zeros(nb, dtype=np.int64)
    for c in range(nc_):
        dl = deg_new[c * n_loc:(c + 1) * n_loc]
        for b in range(nb):
            seg = dl[b * 128:(b + 1) * 128]
            if seg.size:
                D[b] = max(D[b], int(seg.max()))
    off = np.zeros(nb + 1, dtype=np.int64)
    off[1:] = np.cumsum(D)
    SD = int(off[-1])
    TOT = 128 * SD

    # gather-call chunks: consecutive buckets, <= scap slot-columns each
    chunks = []
    b0 = 0
    while b0 < nb:
        b1 = b0
        s = 0
        while b1 < nb and s + D[b1] <= cfg["scap"]:
            s += int(D[b1])
            b1 += 1
        assert b1 > b0, f"bucket {b0} D={D[b0]} exceeds scap={cfg['scap']}"
        chunks.append((b0, b1, int(off[b0]), s))
        b0 = b1

    ZP = n  # row n of the gather table is zeroed (pad slots)

    # edge ranks within destination (vectorized)
    order = np.argsort(dst_n, kind="stable")
    ds, ss = dst_n[order], src_n[order]
    counts = np.bincount(ds, minlength=n)
    starts = np.zeros(n, dtype=np.int64)
    starts[1:] = np.cumsum(counts)[:-1]
    rank = np.arange(len(ds)) - starts[ds]   # 0..indeg-1 per dst

    per_core = []
    for c in range(nc_):
        lo, hi = c * n_loc, (c + 1) * n_loc
        m = (ds >= lo) & (ds < hi)
        dloc = ds[m] - lo
        s_new = ss[m]
        k = rank[m] + 1                      # slot 0 reserved for self loop
        b = dloc // 128
        p = dloc % 128

        # idx32[p, off_b + k] = source row (new id) of dst lane p, slot k
        idx32 = np.full((128, SD), ZP, dtype=np.int32)
        dl_all = np.arange(lo, hi)
        bb = (dl_all - lo) // 128
        pp_ = (dl_all - lo) % 128
        idx32[pp_, off[bb]] = dl_all          # self loops at slot 0
        idx32[p, off[b] + k] = s_new          # edges
        degt = np.ones((128, nb), dtype=np.float32)
        dl = deg_new[lo:hi].astype(np.float32)
        degt[(np.arange(n_loc) % 128), (np.arange(n_loc) // 128)] = dl

        per_core.append(dict(idx32=np.ascontiguousarray(idx32), deg=degt))

    meta = dict(D=[int(x) for x in D], off=off, SD=SD, chunks=chunks,
                perm_old=perm_old, ZP=ZP)
    return meta, per_core




# ------------------------------------------------- toolchain compatibility
# The walrus build in this container predates two instructions that current
# concourse emits: EVENT_SEMAPHORE_RANGE_CLEAR (Tile's bulk semaphore reset)
# and multi-wait sync lists.  Replace the former with per-semaphore
# EventSemaphore writes (sem-wr-imm 0) and split the latter onto preceding
# same-engine EventSemaphore waits.

_PATCHED = False
_COMPAT_ON = False


def _apply_compat_patches():
    global _PATCHED
    if _PATCHED:
        return
    import concourse.bass as bass
    import concourse.mybir as mybir

    orig = bass.BassGpSimd.sem_clear

    def sem_clear_patched(self, sem):
        if not _COMPAT_ON:
            return orig(self, sem)
        ids = list(sem) if isinstance(sem, range) else [sem.num]
        last = None
        for m in ids:
            w = mybir.InstEventSemaphore(
                name=f"semwr_{self.bass.next_id()}", ins=[], outs=[])
            w.sync_info = mybir.SyncInfo(on_wait=[], on_update=[
                mybir.SyncUpdate(sync_type="semaphore", id=m,
                                 update_mode="sem-wr-imm", update_value=0,
                                 ant_name=f"clr{m}")])
            last = self.add_instruction(w)
        return last

    bass.BassGpSimd.sem_clear = sem_clear_patched
    _PATCHED = True


def _split_waits(nc, max_waits=1):
    import concourse.mybir as mybir
    for fn in nc.m.functions:
        for bb in fn.blocks:
            newlist = []
            changed = False
            for ins in list(bb.instructions):
                si = ins.sync_info
                waits = list(si.on_wait) if si is not None and si.on_wait else []
                if len(waits) > max_waits:
                    keep, extra = waits[:max_waits], waits[max_waits:]
                    k = 0
                    while extra:
                        grp, extra = extra[:max_waits], extra[max_waits:]
                        k += 1
                        w = mybir.InstEventSemaphore(
                            name=f"ws_{ins.name}_{k}", ins=[], outs=[])
                        w.engine = ins.engine
                        w.sync_info = mybir.SyncInfo(on_wait=grp, on_update=[])
                        newlist.append(w)
                        changed = True
                    si.on_wait = keep
                newlist.append(ins)
            if changed:
                bb.instructions = newlist


# ------------------------------------------------------------ bass builder

def _build(cfg, meta, compat=True):
    global _COMPAT_ON
    import concourse.bass as bass
    import concourse.mybir as mybir
    import concourse.tile as tile
    from concourse.masks import make_identity

    _apply_compat_patches()
    _COMPAT_ON = compat

    dt = mybir.dt
    DT = dt.bfloat16 if cfg["use_bf16"] else dt.float32
    f32 = dt.float32
    n, n_loc, nb, n_pad = cfg["N"], cfg["n_loc"], cfg["NB"], cfg["n_pad"]
    trows = cfg["TROWS"]
    SD = meta["SD"]
    D = meta["D"]
    off = meta["off"]
    chunks = meta["chunks"]
    rg = [list(range(cfg["NC"]))]

    import contextlib
    ctx = contextlib.ExitStack()
    nc = bass.Bass("TRN2", num_devices=cfg["NC"], num_swdge_queues=2)

    # --- IO ---
    xT = nc.dram_tensor("xT", [F_IN, n_pad], f32, kind="ExternalInput")
    idx_d = nc.dram_tensor("idx32", [128, SD], dt.int32, kind="ExternalInput")
    deg_d = nc.dram_tensor("deg", [128, nb], f32, kind="ExternalInput")
    W_d = [nc.dram_tensor("W0", [F_IN, HID], f32, kind="ExternalInput"),
           nc.dram_tensor("W1", [HID, HID], f32, kind="ExternalInput"),
           nc.dram_tensor("W2", [HID, HID], f32, kind="ExternalInput"),
           nc.dram_tensor("W3", [HID, CDIM], f32, kind="ExternalInput")]
    b_d = [nc.dram_tensor(f"b{i}", [1, HID if i < 3 else CDIM], f32,
                          kind="ExternalInput") for i in range(4)]
    Wl_d = nc.dram_tensor("Wl", [CDIM, KDIM], f32, kind="ExternalInput")
    bl_d = nc.dram_tensor("bl", [1, KDIM], f32, kind="ExternalInput")
    conc_o = nc.dram_tensor("concepts", [n_loc, CDIM], f32, kind="ExternalOutput")
    logi_o = nc.dram_tensor("logits", [n_loc, KDIM], f32, kind="ExternalOutput")

    cc_in = nc.dram_tensor("cc_in", [n_loc, 64], DT, kind="Internal")
    cc_out = nc.dram_tensor("cc_out", [trows, 64], DT, kind="Internal",
                            addr_space="Shared")

    FD = [F_IN, HID, HID, HID]   # layer input dims
    FO = [HID, HID, HID, CDIM]   # layer output dims

    with tile.TileContext(nc) as tc:
        with (
            tc.tile_pool(name="const", bufs=1) as cp,
            tc.tile_pool(name="gath", bufs=GATHER_BUFS) as gp,
            tc.tile_pool(name="work", bufs=3) as wp,
            tc.tile_pool(name="ps", bufs=2, space="PSUM") as pp,
        ):
            # ---------- constants ----------
            idx_sb = cp.tile([128, SD], dt.int32)
            nc.sync.dma_start(idx_sb[:], idx_d[:])
            deg_sb = cp.tile([128, nb], f32)
            nc.sync.dma_start(deg_sb[:], deg_d[:])
            dinv_sb = cp.tile([128, nb], f32)
            nc.scalar.activation(dinv_sb[:], deg_sb[:],
                                 mybir.ActivationFunctionType.Sqrt)
            nc.vector.reciprocal(dinv_sb[:], dinv_sb[:])

            ident = cp.tile([128, 128], f32)
            make_identity(nc, ident[:])
            ones1 = cp.tile([1, 128], f32)
            nc.vector.memset(ones1[:], 1.0)

            xT0_sb = cp.tile([128, n_pad], f32)
            nc.sync.dma_start(xT0_sb[:], xT[:])

            W_sb = []
            for i in range(4):
                t = cp.tile([FD[i], FO[i]], f32, tag=f"w{i}")
                nc.sync.dma_start(t[:], W_d[i][:])
                W_sb.append(t)
            Wl_sb = cp.tile([CDIM, KDIM], f32)
            nc.sync.dma_start(Wl_sb[:], Wl_d[:])

            # broadcast biases to all partitions via rank-1 matmul
            b_bc = []
            for i in range(4):
                w = FO[i]
                bs = cp.tile([1, w], f32, tag=f"bld{i}")
                nc.sync.dma_start(bs[:], b_d[i][:])
                ps = pp.tile([128, w], f32, tag="mm")
                nc.tensor.matmul(ps[:], lhsT=ones1[:], rhs=bs[:],
                                 start=True, stop=True)
                bb = cp.tile([128, w], f32, tag=f"bbc{i}")
                nc.scalar.copy(bb[:], ps[:])
                b_bc.append(bb)
            bls = cp.tile([1, KDIM], f32)
            nc.sync.dma_start(bls[:], bl_d[:])
            psl = pp.tile([128, KDIM], f32, tag="mm")
            nc.tensor.matmul(psl[:], lhsT=ones1[:], rhs=bls[:],
                             start=True, stop=True)
            bl_bc = cp.tile([128, KDIM], f32)
            nc.scalar.copy(bl_bc[:], psl[:])

            # zero the pad rows of the gather table
            zt = cp.tile([128, 64], DT)
            nc.vector.memset(zt[:], 0.0)
            nc.sync.dma_start(cc_out[n:n + 128, :], zt[:])

            # persistent per-layer node features [lane, block*64]
            feat = cp.tile([128, nb * 64], f32)

            # ---------- layers ----------
            for l in range(4):
                fi, fo = FD[l], FO[l]
                # phase A: hs = dinv * (input @ W) -> cc_in
                for b in range(nb):
                    nrow = min(128, n_loc - b * 128)
                    if l == 0:
                        lhs = xT0_sb[:, b * 128:(b + 1) * 128]
                    else:
                        pt = pp.tile([64, 128], f32, tag="tr")
                        nc.tensor.transpose(
                            pt[:], feat[:, b * 64:b * 64 + 64], ident[:])
                        xtb = wp.tile([64, 128], f32, tag="xtb")
                        nc.scalar.copy(xtb[:], pt[:])
                        lhs = xtb[:fi, :]
                    hp = pp.tile([128, fo], f32, tag="mm")
                    nc.tensor.matmul(hp[:], lhsT=lhs, rhs=W_sb[l][:],
                                     start=True, stop=True)
                    hs = wp.tile([128, 64], DT, tag="hs")
                    if fo < 64:
                        nc.vector.memset(hs[:], 0.0)
                    nc.scalar.activation(hs[:, :fo], hp[:],
                                         mybir.ActivationFunctionType.Copy,
                                         scale=dinv_sb[:, b:b + 1])
                    nc.sync.dma_start(cc_in[b * 128:b * 128 + nrow, :],
                                      hs[:nrow, :])

                # phase B: AllGather into the shared table
                nc.gpsimd.collective_compute(
                    "AllGather", mybir.AluOpType.bypass, replica_groups=rg,
                    ins=[cc_in[:, :].opt()], outs=[cc_out[:n, :].opt()])

                # phase C: gather + select + segmented reduce + post
                for ci, (b0, b1, off0, S) in enumerate(chunks):
                    g = gp.tile([128, S, 64], DT, tag="g")
                    for j in range(S):
                        nc.gpsimd.indirect_dma_start(
                            out=g[:, j, :], out_offset=None,
                            in_=cc_out[:],
                            in_offset=bass.IndirectOffsetOnAxis(
                                ap=idx_sb[:, off0 + j:off0 + j + 1], axis=0))

                    o = 0
                    for b in range(b0, b1):
                        Db = D[b]
                        agg = wp.tile([128, 64], f32, tag="agg")
                        nc.vector.reduce_sum(
                            agg[:], g[:, o:o + Db, :].rearrange(
                                "p s f -> p f s"),
                            axis=mybir.AxisListType.X)
                        t1 = wp.tile([128, 64], f32, tag="t1")
                        nc.scalar.activation(
                            t1[:, :fo], agg[:, :fo],
                            mybir.ActivationFunctionType.Copy,
                            scale=dinv_sb[:, b:b + 1])
                        nc.vector.tensor_add(t1[:, :fo], t1[:, :fo],
                                             b_bc[l][:, :fo])
                        t2 = wp.tile([128, 64], f32, tag="t2")
                        nc.vector.tensor_scalar_mul(t2[:, :fo], t1[:, :fo], 0.01)
                        nc.vector.tensor_tensor(
                            out=feat[:, b * 64:b * 64 + fo],
                            in0=t1[:, :fo], in1=t2[:, :fo],
                            op=mybir.AluOpType.max)
                        o += Db

            # ---------- head ----------
            for b in range(nb):
                nrow = min(128, n_loc - b * 128)
                f3 = feat[:, b * 64:b * 64 + CDIM]
                mx = wp.tile([128, 1], f32, tag="mx")
                nc.vector.reduce_max(mx[:], f3, axis=mybir.AxisListType.X)
                nmx = wp.tile([128, 1], f32, tag="nmx")
                nc.vector.tensor_scalar_mul(nmx[:], mx[:], -1.0)
                cc = wp.tile([128, CDIM], f32, tag="cc")
                nc.scalar.activation(cc[:], f3,
                                     mybir.ActivationFunctionType.Exp,
                                     bias=nmx[:, :1])
                nc.sync.dma_start(conc_o[b * 128:b * 128 + nrow, :],
                                  cc[:nrow, :])
                ptc = pp.tile([CDIM, 128], f32, tag="tr")
                nc.tensor.transpose(ptc[:], cc[:], ident[:])
                ct = wp.tile([CDIM, 128], f32, tag="ct")
                nc.scalar.copy(ct[:], ptc[:])
                lg = pp.tile([128, KDIM], f32, tag="mm")
                nc.tensor.matmul(lg[:], lhsT=ct[:], rhs=Wl_sb[:],
                                 start=True, stop=True)
                lgs = wp.tile([128, KDIM], f32, tag="lgs")
                nc.vector.tensor_add(lgs[:], lg[:], bl_bc[:])
                nc.sync.dma_start(logi_o[b * 128:b * 128 + nrow, :],
                                  lgs[:nrow, :])

    ctx.close()
    _COMPAT_ON = False
    if compat:
        _split_waits(nc, 1)
    return nc


# ---------------------------------------------------------------- kernel

def _make_in_maps(cfg, meta, per_core, x, Ws, bs, Wl, bl):
    perm_old = meta["perm_old"]
    n_loc, n_pad = cfg["n_loc"], cfg["n_pad"]
    np_dt = ml_dtypes.bfloat16 if cfg["use_bf16"] else np.float32
    in_maps = []
    for c in range(cfg["NC"]):
        rows = perm_old[c * n_loc:(c + 1) * n_loc]
        xTl = np.zeros((F_IN, n_pad), dtype=np.float32)
        xTl[:, :n_loc] = x[rows].T
        im = dict(
            xT=np.ascontiguousarray(xTl),
            idx32=per_core[c]["idx32"],
            deg=per_core[c]["deg"],
            W0=Ws[0], W1=Ws[1], W2=Ws[2], W3=Ws[3],
            b0=bs[0].reshape(1, -1), b1=bs[1].reshape(1, -1),
            b2=bs[2].reshape(1, -1), b3=bs[3].reshape(1, -1),
            Wl=Wl, bl=bl.reshape(1, -1),
        )
        in_maps.append({k: np.ascontiguousarray(v) for k, v in im.items()})
    return in_maps


def _run_pjrt(nc, in_maps, n_cores, benchmark=0):
    """Execute via PJRT/axon (mirrors bass2jax.run_bass_via_pjrt) with an
    optional repeated-execution benchmark on device-resident inputs."""
    import time
    import jax
    import concourse.mybir as mybir
    from concourse import bass2jax
    from concourse.bass2jax import _bass_exec_p, partition_id_tensor
    from jax.sharding import Mesh, PartitionSpec, NamedSharding
    from jax.experimental.shard_map import shard_map

    bass2jax.install_neuronx_cc_hook()

    partition_name = (nc.partition_id_tensor.name
                      if nc.partition_id_tensor else None)
    in_names, out_names, out_avals, zero_outs = [], [], [], []
    for alloc in nc.m.functions[0].allocations:
        if not isinstance(alloc, mybir.MemoryLocationSet):
            continue
        name = alloc.memorylocations[0].name
        if alloc.kind == "ExternalInput":
            if name != partition_name:
                in_names.append(name)
        elif alloc.kind == "ExternalOutput":
            out_names.append(name)
            shape = tuple(alloc.tensor_shape)
            dtype = mybir.dt.np(alloc.dtype)
            out_avals.append(jax.core.ShapedArray(shape, dtype))
            zero_outs.append(np.zeros(shape, dtype))
    n_params = len(in_names)
    in_names = in_names + out_names
    if partition_name is not None:
        in_names.append(partition_name)

    def _body(*args):
        operands = list(args)
        if partition_name is not None:
            operands.append(partition_id_tensor())
        outs = _bass_exec_p.bind(
            *operands, out_avals=tuple(out_avals), in_names=tuple(in_names),
            out_names=tuple(out_names), lowering_input_output_aliases=(),
            sim_require_finite=True, sim_require_nnan=True, nc=nc)
        return tuple(outs)

    devices = jax.devices()[:n_cores]
    mesh = Mesh(np.asarray(devices), ("core",))
    nin = n_params + len(out_names)
    sharded = jax.jit(
        shard_map(_body, mesh=mesh,
                  in_specs=(PartitionSpec("core"),) * nin,
                  out_specs=(PartitionSpec("core"),) * len(out_names),
                  check_rep=False),
        keep_unused=True)

    sh = NamedSharding(mesh, PartitionSpec("core"))
    concat_in = [
        jax.device_put(
            np.concatenate([np.asarray(in_maps[c][in_names[i]])
                            for c in range(n_cores)], axis=0), sh)
        for i in range(n_params)
    ] + [jax.device_put(np.concatenate([z] * n_cores, axis=0), sh)
         for z in zero_outs]

    outs = jax.block_until_ready(sharded(*concat_in))
    bench_ns = None
    if benchmark:
        times = []
        for _ in range(benchmark):
            t0 = time.perf_counter()
            o = jax.block_until_ready(sharded(*concat_in))
            times.append(time.perf_counter() - t0)
        bench_ns = int(min(times) * 1e9)
        del o

    results = []
    for c in range(n_cores):
        d = {}
        for i, name in enumerate(out_names):
            full = np.asarray(outs[i])
            per = full.shape[0] // n_cores
            d[name] = full[c * per:(c + 1) * per]
        results.append(d)
    return results, bench_ns


_CACHE = {}


def kernel(x, src, dst, W0, b0, W1, b1, W2, b2, W3, b3, Wl, bl,
           benchmark=0, **_ignored):
    x = np.asarray(x, dtype=np.float32)
    Ws = [np.asarray(w, dtype=np.float32) for w in (W0, W1, W2, W3)]
    bs = [np.asarray(b, dtype=np.float32) for b in (b0, b1, b2, b3)]
    Wl = np.asarray(Wl, dtype=np.float32)
    bl = np.asarray(bl, dtype=np.float32)

    key = (int(np.asarray(src[:64]).sum()), int(np.asarray(dst[:64]).sum()),
           len(np.asarray(src)))
    if key in _CACHE:
        cfg, meta, per_core, nc = _CACHE[key]
    else:
        cfg = _cfg(N, E, N_CORES)
        meta, per_core = _preprocess(cfg, src, dst)
        nc = _build(cfg, meta)
        _CACHE[key] = (cfg, meta, per_core, nc)

    in_maps = _make_in_maps(cfg, meta, per_core, x, Ws, bs, Wl, bl)
    results, bench_ns = _run_pjrt(nc, in_maps, cfg["NC"], benchmark=benchmark)
    kernel.bench_ns = bench_ns

    perm_old = meta["perm_old"]
    concepts = np.empty((N, CDIM), dtype=np.float32)
    logits = np.empty((N, KDIM), dtype=np.float32)
    n_loc = cfg["n_loc"]
    for c in range(cfg["NC"]):
        rows = perm_old[c * n_loc:(c + 1) * n_loc]
        concepts[rows] = results[c]["concepts"]
        logits[rows] = results[c]["logits"]
    return concepts, logits


# revision 19
# speedup vs baseline: 1.3303x; 1.3303x over previous
"""GCN (4-layer, BA-Shapes-style) Trainium2 kernel, 8 NeuronCores.

Strategy (graph/data parallel, per sharding hint):
  - Nodes are sharded into 8 contiguous ranges (6250 nodes/core). Each core
    owns the aggregation for its node range.
  - Per layer: each core computes hs = dinv * (x @ W) for its own rows,
    AllGathers hs into a replicated DRAM table, then PULL-aggregates: for
    every owned destination node, indirect DMA fetches the hs rows of its
    in-neighbors and a Vector-engine segmented reduce sums them; the self
    contribution is added from SBUF.  out = leaky_relu(dinv * agg + b).
  - GCN normalization deg^-1/2 factorizes to a row scale before the gather
    and a row scale after the reduce, so no per-edge weights are needed.
  - Head: concepts = softmax(h)/max(softmax(h)) == exp(h - rowmax(h));
    logits = concepts @ Wl + bl.

Host-side work is restricted to integer index preprocessing (edge lists,
degree counts, permutations, padding plans) and input/output layout
(transpose / permutation of rows); all floating-point math runs on device.

Index-layout details:
  - The gather uses per-slot-column indirect DMA (offsets [128,1] int32,
    one gathered row per partition) -- the one indirect form the installed
    toolchain lowers correctly.  Self-loop terms are added from SBUF.
  - Destinations are bucketed 128-at-a-time by in-degree rank; every bucket
    is padded to a cross-core-uniform slot count D[b] so all 8 cores share
    one NEFF.  Pad slots gather a zeroed table row (contributing 0).
"""

import numpy as np
import ml_dtypes

# ----------------------------------------------------------------- config

N_CORES = 8
N, E = 50000, 800000
F_IN, HID, CDIM, KDIM = 128, 64, 32, 4

SCAP = 64          # max slot-columns per dma_gather call (num_idxs = 128*SCAP)
GATHER_BUFS = 3
USE_BF16 = True    # table/gather/select dtype (reduce always accumulates fp32)


def _cfg(n, e, n_cores, use_bf16=USE_BF16, scap=SCAP):
    n_loc = n // n_cores
    nb = (n_loc + 127) // 128
    return dict(
        N=n, E=e, NC=n_cores, n_loc=n_loc, NB=nb, n_pad=nb * 128,
        TROWS=n + 128,            # extra zeroed rows for pad slots
        use_bf16=use_bf16, scap=scap,
    )


# ----------------------------------------------------- host preprocessing

def _preprocess(cfg, src, dst):
    """Integer-only index preprocessing.

    Returns (cfg additions, per-core arrays):
      perm_old[new] : global node permutation (core-range, degree-sorted)
      D[b]          : uniform bucket slot counts
      chunks        : [(b0, b1, off0, S)] gather-call partition
      per core: idx16 [128, 8*SD] int16, maskL [128, SD], deg [128, NB] f32
    """
    n, nc_, n_loc, nb = cfg["N"], cfg["NC"], cfg["n_loc"], cfg["NB"]
    src = np.asarray(src, dtype=np.int64)
    dst = np.asarray(dst, dtype=np.int64)

    indeg = np.bincount(dst, minlength=n)
    deg = indeg + 1  # self loop (for normalization; self handled on-chip)

    # permutation: within each core's contiguous range, sort nodes by degree
    perm_old = np.empty(n, dtype=np.int64)   # new -> old
    for c in range(nc_):
        lo = c * n_loc
        rng = np.arange(lo, lo + n_loc)
        order = np.argsort(deg[rng], kind="stable")
        perm_old[lo:lo + n_loc] = rng[order]
    new_of_old = np.empty(n, dtype=np.int64)
    new_of_old[perm_old] = np.arange(n)

    src_n = new_of_old[src]
    dst_n = new_of_old[dst]
    deg_new = deg[perm_old]                  # degree in new id order

    # bucket profile: D[b] = max over cores of bucket max IN-degree
    # (self loops are added from SBUF, not gathered)
    D = np.zeros(nb, dtype=np.int64)
    for c in range(nc_):
        dl = deg_new[c * n_loc:(c + 1) * n_loc] - 1
        for b in range(nb):
            seg = dl[b * 128:(b + 1) * 128]
            if seg.size:
                D[b] = max(D[b], int(seg.max()))
    D = np.maximum(D, 1)
    off = np.zeros(nb + 1, dtype=np.int64)
    off[1:] = np.cumsum(D)
    SD = int(off[-1])
    TOT = 128 * SD

    # gather-call chunks: consecutive buckets, <= scap slot-columns each
    chunks = []
    b0 = 0
    while b0 < nb:
        b1 = b0
        s = 0
        while b1 < nb and s + D[b1] <= cfg["scap"]:
            s += int(D[b1])
            b1 += 1
        assert b1 > b0, f"bucket {b0} D={D[b0]} exceeds scap={cfg['scap']}"
        chunks.append((b0, b1, int(off[b0]), s))
        b0 = b1

    ZP = n  # row n of the gather table is zeroed (pad slots)

    # edge ranks within destination (vectorized)
    order = np.argsort(dst_n, kind="stable")
    ds, ss = dst_n[order], src_n[order]
    counts = np.bincount(ds, minlength=n)
    starts = np.zeros(n, dtype=np.int64)
    starts[1:] = np.cumsum(counts)[:-1]
    rank = np.arange(len(ds)) - starts[ds]   # 0..indeg-1 per dst

    per_core = []
    for c in range(nc_):
        lo, hi = c * n_loc, (c + 1) * n_loc
        m = (ds >= lo) & (ds < hi)
        dloc = ds[m] - lo
        s_new = ss[m]
        k = rank[m]                          # in-edges only; self added on-chip
        b = dloc // 128
        p = dloc % 128

        # idx32[p, off_b + k] = source row (new id) of dst lane p, slot k
        idx32 = np.full((128, SD), ZP, dtype=np.int32)
        idx32[p, off[b] + k] = s_new          # edges
        degt = np.ones((128, nb), dtype=np.float32)
        dl = deg_new[lo:hi].astype(np.float32)
        degt[(np.arange(n_loc) % 128), (np.arange(n_loc) // 128)] = dl

        per_core.append(dict(idx32=np.ascontiguousarray(idx32), deg=degt))

    meta = dict(D=[int(x) for x in D], off=off, SD=SD, chunks=chunks,
                perm_old=perm_old, ZP=ZP)
    return meta, per_core




# ------------------------------------------------- toolchain compatibility
# The walrus build in this container predates two instructions that current
# concourse emits: EVENT_SEMAPHORE_RANGE_CLEAR (Tile's bulk semaphore reset)
# and multi-wait sync lists.  Replace the former with per-semaphore
# EventSemaphore writes (sem-wr-imm 0) and split the latter onto preceding
# same-engine EventSemaphore waits.

_PATCHED = False
_COMPAT_ON = False


def _apply_compat_patches():
    global _PATCHED
    if _PATCHED:
        return
    import concourse.bass as bass
    import concourse.mybir as mybir

    orig = bass.BassGpSimd.sem_clear

    def sem_clear_patched(self, sem):
        if not _COMPAT_ON:
            return orig(self, sem)
        ids = list(sem) if isinstance(sem, range) else [sem.num]
        last = None
        for m in ids:
            w = mybir.InstEventSemaphore(
                name=f"semwr_{self.bass.next_id()}", ins=[], outs=[])
            w.sync_info = mybir.SyncInfo(on_wait=[], on_update=[
                mybir.SyncUpdate(sync_type="semaphore", id=m,
                                 update_mode="sem-wr-imm", update_value=0,
                                 ant_name=f"clr{m}")])
            last = self.add_instruction(w)
        return last

    bass.BassGpSimd.sem_clear = sem_clear_patched
    _PATCHED = True


def _split_waits(nc, max_waits=1):
    import concourse.mybir as mybir
    for fn in nc.m.functions:
        for bb in fn.blocks:
            newlist = []
            changed = False
            for ins in list(bb.instructions):
                si = ins.sync_info
                waits = list(si.on_wait) if si is not None and si.on_wait else []
                if len(waits) > max_waits:
                    keep, extra = waits[:max_waits], waits[max_waits:]
                    k = 0
                    while extra:
                        grp, extra = extra[:max_waits], extra[max_waits:]
                        k += 1
                        w = mybir.InstEventSemaphore(
                            name=f"ws_{ins.name}_{k}", ins=[], outs=[])
                        w.engine = ins.engine
                        w.sync_info = mybir.SyncInfo(on_wait=grp, on_update=[])
                        newlist.append(w)
                        changed = True
                    si.on_wait = keep
                newlist.append(ins)
            if changed:
                bb.instructions = newlist


# ------------------------------------------------------------ bass builder

def _build(cfg, meta, compat=True):
    global _COMPAT_ON
    import concourse.bass as bass
    import concourse.mybir as mybir
    import concourse.tile as tile
    from concourse.masks import make_identity

    _apply_compat_patches()
    _COMPAT_ON = compat

    dt = mybir.dt
    DT = dt.bfloat16 if cfg["use_bf16"] else dt.float32
    f32 = dt.float32
    n, n_loc, nb, n_pad = cfg["N"], cfg["n_loc"], cfg["NB"], cfg["n_pad"]
    trows = cfg["TROWS"]
    SD = meta["SD"]
    D = meta["D"]
    off = meta["off"]
    chunks = meta["chunks"]
    rg = [list(range(cfg["NC"]))]

    import contextlib
    ctx = contextlib.ExitStack()
    nc = bass.Bass("TRN2", num_devices=cfg["NC"], num_swdge_queues=2)

    # --- IO ---
    xT = nc.dram_tensor("xT", [F_IN, n_pad], f32, kind="ExternalInput")
    idx_d = nc.dram_tensor("idx32", [128, SD], dt.int32, kind="ExternalInput")
    deg_d = nc.dram_tensor("deg", [128, nb], f32, kind="ExternalInput")
    W_d = [nc.dram_tensor("W0", [F_IN, HID], f32, kind="ExternalInput"),
           nc.dram_tensor("W1", [HID, HID], f32, kind="ExternalInput"),
           nc.dram_tensor("W2", [HID, HID], f32, kind="ExternalInput"),
           nc.dram_tensor("W3", [HID, CDIM], f32, kind="ExternalInput")]
    b_d = [nc.dram_tensor(f"b{i}", [1, HID if i < 3 else CDIM], f32,
                          kind="ExternalInput") for i in range(4)]
    Wl_d = nc.dram_tensor("Wl", [CDIM, KDIM], f32, kind="ExternalInput")
    bl_d = nc.dram_tensor("bl", [1, KDIM], f32, kind="ExternalInput")
    conc_o = nc.dram_tensor("concepts", [n_loc, CDIM], f32, kind="ExternalOutput")
    logi_o = nc.dram_tensor("logits", [n_loc, KDIM], f32, kind="ExternalOutput")

    cc_in = nc.dram_tensor("cc_in", [n_loc, 64], DT, kind="Internal")
    cc_out = nc.dram_tensor("cc_out", [trows, 64], DT, kind="Internal",
                            addr_space="Shared")

    FD = [F_IN, HID, HID, HID]   # layer input dims
    FO = [HID, HID, HID, CDIM]   # layer output dims

    with tile.TileContext(nc) as tc:
        with (
            tc.tile_pool(name="const", bufs=1) as cp,
            tc.tile_pool(name="gath", bufs=GATHER_BUFS) as gp,
            tc.tile_pool(name="work", bufs=3) as wp,
            tc.tile_pool(name="ps", bufs=2, space="PSUM") as pp,
        ):
            # ---------- constants ----------
            idx_sb = cp.tile([128, SD], dt.int32)
            nc.sync.dma_start(idx_sb[:], idx_d[:])
            deg_sb = cp.tile([128, nb], f32)
            nc.sync.dma_start(deg_sb[:], deg_d[:])
            dinv_sb = cp.tile([128, nb], f32)
            nc.scalar.activation(dinv_sb[:], deg_sb[:],
                                 mybir.ActivationFunctionType.Sqrt)
            nc.vector.reciprocal(dinv_sb[:], dinv_sb[:])

            ident = cp.tile([128, 128], f32)
            make_identity(nc, ident[:])
            ones1 = cp.tile([1, 128], f32)
            nc.vector.memset(ones1[:], 1.0)

            xT0_sb = cp.tile([128, n_pad], f32)
            nc.sync.dma_start(xT0_sb[:], xT[:])

            W_sb = []
            for i in range(4):
                t = cp.tile([FD[i], FO[i]], f32, tag=f"w{i}")
                nc.sync.dma_start(t[:], W_d[i][:])
                W_sb.append(t)
            Wl_sb = cp.tile([CDIM, KDIM], f32)
            nc.sync.dma_start(Wl_sb[:], Wl_d[:])

            # broadcast biases to all partitions via rank-1 matmul
            b_bc = []
            for i in range(4):
                w = FO[i]
                bs = cp.tile([1, w], f32, tag=f"bld{i}")
                nc.sync.dma_start(bs[:], b_d[i][:])
                ps = pp.tile([128, w], f32, tag="mm")
                nc.tensor.matmul(ps[:], lhsT=ones1[:], rhs=bs[:],
                                 start=True, stop=True)
                bb = cp.tile([128, w], f32, tag=f"bbc{i}")
                nc.scalar.copy(bb[:], ps[:])
                b_bc.append(bb)
            bls = cp.tile([1, KDIM], f32)
            nc.sync.dma_start(bls[:], bl_d[:])
            psl = pp.tile([128, KDIM], f32, tag="mm")
            nc.tensor.matmul(psl[:], lhsT=ones1[:], rhs=bls[:],
                             start=True, stop=True)
            bl_bc = cp.tile([128, KDIM], f32)
            nc.scalar.copy(bl_bc[:], psl[:])

            # zero the pad rows of the gather table
            zt = cp.tile([128, 64], DT)
            nc.vector.memset(zt[:], 0.0)
            nc.sync.dma_start(cc_out[n:n + 128, :], zt[:])

            # persistent per-layer node features [lane, block*64]
            feat = cp.tile([128, nb * 64], f32)
            # persistent scaled features hs = dinv*h (self-loop contribution)
            hs_sb = cp.tile([128, nb * 64], DT)

            # ---------- layers ----------
            for l in range(4):
                fi, fo = FD[l], FO[l]
                # phase A: hs = dinv * (input @ W) -> cc_in
                for b in range(nb):
                    nrow = min(128, n_loc - b * 128)
                    if l == 0:
                        lhs = xT0_sb[:, b * 128:(b + 1) * 128]
                    else:
                        pt = pp.tile([64, 128], f32, tag="tr")
                        nc.tensor.transpose(
                            pt[:], feat[:, b * 64:b * 64 + 64], ident[:])
                        xtb = wp.tile([64, 128], f32, tag="xtb")
                        nc.scalar.copy(xtb[:], pt[:])
                        lhs = xtb[:fi, :]
                    hp = pp.tile([128, fo], f32, tag="mm")
                    nc.tensor.matmul(hp[:], lhsT=lhs, rhs=W_sb[l][:],
                                     start=True, stop=True)
                    hs = hs_sb[:, b * 64:(b + 1) * 64]
                    if fo < 64:
                        nc.vector.memset(hs, 0.0)
                    nc.scalar.activation(hs[:, :fo], hp[:],
                                         mybir.ActivationFunctionType.Copy,
                                         scale=dinv_sb[:, b:b + 1])
                    nc.sync.dma_start(cc_in[b * 128:b * 128 + nrow, :],
                                      hs[:nrow, :])

                # phase B: AllGather into the shared table
                nc.gpsimd.collective_compute(
                    "AllGather", mybir.AluOpType.bypass, replica_groups=rg,
                    ins=[cc_in[:, :].opt()], outs=[cc_out[:n, :].opt()])

                # phase C: gather + select + segmented reduce + post
                for ci, (b0, b1, off0, S) in enumerate(chunks):
                    g = gp.tile([128, S, 64], DT, tag="g")
                    for j in range(S):
                        nc.gpsimd.indirect_dma_start(
                            out=g[:, j, :], out_offset=None,
                            in_=cc_out[:],
                            in_offset=bass.IndirectOffsetOnAxis(
                                ap=idx_sb[:, off0 + j:off0 + j + 1], axis=0))

                    o = 0
                    for b in range(b0, b1):
                        Db = D[b]
                        agg = wp.tile([128, 64], f32, tag="agg")
                        nc.vector.reduce_sum(
                            agg[:], g[:, o:o + Db, :].rearrange(
                                "p s f -> p f s"),
                            axis=mybir.AxisListType.X)
                        nc.vector.tensor_add(
                            agg[:, :fo], agg[:, :fo],
                            hs_sb[:, b * 64:b * 64 + fo])
                        t1 = wp.tile([128, 64], f32, tag="t1")
                        nc.scalar.activation(
                            t1[:, :fo], agg[:, :fo],
                            mybir.ActivationFunctionType.Copy,
                            scale=dinv_sb[:, b:b + 1])
                        nc.vector.tensor_add(t1[:, :fo], t1[:, :fo],
                                             b_bc[l][:, :fo])
                        t2 = wp.tile([128, 64], f32, tag="t2")
                        nc.vector.tensor_scalar_mul(t2[:, :fo], t1[:, :fo], 0.01)
                        nc.vector.tensor_tensor(
                            out=feat[:, b * 64:b * 64 + fo],
                            in0=t1[:, :fo], in1=t2[:, :fo],
                            op=mybir.AluOpType.max)
                        o += Db

            # ---------- head ----------
            for b in range(nb):
                nrow = min(128, n_loc - b * 128)
                f3 = feat[:, b * 64:b * 64 + CDIM]
                mx = wp.tile([128, 1], f32, tag="mx")
                nc.vector.reduce_max(mx[:], f3, axis=mybir.AxisListType.X)
                nmx = wp.tile([128, 1], f32, tag="nmx")
                nc.vector.tensor_scalar_mul(nmx[:], mx[:], -1.0)
                cc = wp.tile([128, CDIM], f32, tag="cc")
                nc.scalar.activation(cc[:], f3,
                                     mybir.ActivationFunctionType.Exp,
                                     bias=nmx[:, :1])
                nc.sync.dma_start(conc_o[b * 128:b * 128 + nrow, :],
                                  cc[:nrow, :])
                ptc = pp.tile([CDIM, 128], f32, tag="tr")
                nc.tensor.transpose(ptc[:], cc[:], ident[:])
                ct = wp.tile([CDIM, 128], f32, tag="ct")
                nc.scalar.copy(ct[:], ptc[:])
                lg = pp.tile([128, KDIM], f32, tag="mm")
                nc.tensor.matmul(lg[:], lhsT=ct[:], rhs=Wl_sb[:],
                                 start=True, stop=True)
                lgs = wp.tile([128, KDIM], f32, tag="lgs")
                nc.vector.tensor_add(lgs[:], lg[:], bl_bc[:])
                nc.sync.dma_start(logi_o[b * 128:b * 128 + nrow, :],
                                  lgs[:nrow, :])

    ctx.close()
    _COMPAT_ON = False
    if compat:
        _split_waits(nc, 1)
    return nc


# ---------------------------------------------------------------- kernel

def _make_in_maps(cfg, meta, per_core, x, Ws, bs, Wl, bl):
    perm_old = meta["perm_old"]
    n_loc, n_pad = cfg["n_loc"], cfg["n_pad"]
    np_dt = ml_dtypes.bfloat16 if cfg["use_bf16"] else np.float32
    in_maps = []
    for c in range(cfg["NC"]):
        rows = perm_old[c * n_loc:(c + 1) * n_loc]
        xTl = np.zeros((F_IN, n_pad), dtype=np.float32)
        xTl[:, :n_loc] = x[rows].T
        im = dict(
            xT=np.ascontiguousarray(xTl),
            idx32=per_core[c]["idx32"],
            deg=per_core[c]["deg"],
            W0=Ws[0], W1=Ws[1], W2=Ws[2], W3=Ws[3],
            b0=bs[0].reshape(1, -1), b1=bs[1].reshape(1, -1),
            b2=bs[2].reshape(1, -1), b3=bs[3].reshape(1, -1),
            Wl=Wl, bl=bl.reshape(1, -1),
        )
        in_maps.append({k: np.ascontiguousarray(v) for k, v in im.items()})
    return in_maps


def _run_pjrt(nc, in_maps, n_cores, benchmark=0):
    """Execute via PJRT/axon (mirrors bass2jax.run_bass_via_pjrt) with an
    optional repeated-execution benchmark on device-resident inputs."""
    import time
    import jax
    import concourse.mybir as mybir
    from concourse import bass2jax
    from concourse.bass2jax import _bass_exec_p, partition_id_tensor
    from jax.sharding import Mesh, PartitionSpec, NamedSharding
    from jax.experimental.shard_map import shard_map

    bass2jax.install_neuronx_cc_hook()

    partition_name = (nc.partition_id_tensor.name
                      if nc.partition_id_tensor else None)
    in_names, out_names, out_avals, zero_outs = [], [], [], []
    for alloc in nc.m.functions[0].allocations:
        if not isinstance(alloc, mybir.MemoryLocationSet):
            continue
        name = alloc.memorylocations[0].name
        if alloc.kind == "ExternalInput":
            if name != partition_name:
                in_names.append(name)
        elif alloc.kind == "ExternalOutput":
            out_names.append(name)
            shape = tuple(alloc.tensor_shape)
            dtype = mybir.dt.np(alloc.dtype)
            out_avals.append(jax.core.ShapedArray(shape, dtype))
            zero_outs.append(np.zeros(shape, dtype))
    n_params = len(in_names)
    in_names = in_names + out_names
    if partition_name is not None:
        in_names.append(partition_name)

    def _body(*args):
        operands = list(args)
        if partition_name is not None:
            operands.append(partition_id_tensor())
        outs = _bass_exec_p.bind(
            *operands, out_avals=tuple(out_avals), in_names=tuple(in_names),
            out_names=tuple(out_names), lowering_input_output_aliases=(),
            sim_require_finite=True, sim_require_nnan=True, nc=nc)
        return tuple(outs)

    devices = jax.devices()[:n_cores]
    mesh = Mesh(np.asarray(devices), ("core",))
    nin = n_params + len(out_names)
    sharded = jax.jit(
        shard_map(_body, mesh=mesh,
                  in_specs=(PartitionSpec("core"),) * nin,
                  out_specs=(PartitionSpec("core"),) * len(out_names),
                  check_rep=False),
        keep_unused=True)

    sh = NamedSharding(mesh, PartitionSpec("core"))
    concat_in = [
        jax.device_put(
            np.concatenate([np.asarray(in_maps[c][in_names[i]])
                            for c in range(n_cores)], axis=0), sh)
        for i in range(n_params)
    ] + [jax.device_put(np.concatenate([z] * n_cores, axis=0), sh)
         for z in zero_outs]

    outs = jax.block_until_ready(sharded(*concat_in))
    bench_ns = None
    if benchmark:
        times = []
        for _ in range(benchmark):
            t0 = time.perf_counter()
            o = jax.block_until_ready(sharded(*concat_in))
            times.append(time.perf_counter() - t0)
        bench_ns = int(min(times) * 1e9)
        del o

    results = []
    for c in range(n_cores):
        d = {}
        for i, name in enumerate(out_names):
            full = np.asarray(outs[i])
            per = full.shape[0] // n_cores
            d[name] = full[c * per:(c + 1) * per]
        results.append(d)
    return results, bench_ns


_CACHE = {}


def kernel(x, src, dst, W0, b0, W1, b1, W2, b2, W3, b3, Wl, bl,
           benchmark=0, **_ignored):
    x = np.asarray(x, dtype=np.float32)
    Ws = [np.asarray(w, dtype=np.float32) for w in (W0, W1, W2, W3)]
    bs = [np.asarray(b, dtype=np.float32) for b in (b0, b1, b2, b3)]
    Wl = np.asarray(Wl, dtype=np.float32)
    bl = np.asarray(bl, dtype=np.float32)

    key = (int(np.asarray(src[:64]).sum()), int(np.asarray(dst[:64]).sum()),
           len(np.asarray(src)))
    if key in _CACHE:
        cfg, meta, per_core, nc = _CACHE[key]
    else:
        cfg = _cfg(N, E, N_CORES)
        meta, per_core = _preprocess(cfg, src, dst)
        nc = _build(cfg, meta)
        _CACHE[key] = (cfg, meta, per_core, nc)

    in_maps = _make_in_maps(cfg, meta, per_core, x, Ws, bs, Wl, bl)
    results, bench_ns = _run_pjrt(nc, in_maps, cfg["NC"], benchmark=benchmark)
    kernel.bench_ns = bench_ns

    perm_old = meta["perm_old"]
    concepts = np.empty((N, CDIM), dtype=np.float32)
    logits = np.empty((N, KDIM), dtype=np.float32)
    n_loc = cfg["n_loc"]
    for c in range(cfg["NC"]):
        rows = perm_old[c * n_loc:(c + 1) * n_loc]
        concepts[rows] = results[c]["concepts"]
        logits[rows] = results[c]["logits"]
    return concepts, logits
